# revision 96
# baseline (speedup 1.0000x reference)
"""AdaLN-Zero transformer block (DiT-style) on 8 TRN2 NeuronCores.

Sharding: token-parallel. Core c handles batch b=c//4, sequence block
j=c%4 (S_LOC tokens). The AdaLN modulation matmul is column-sharded
4-ways inside each batch group and all-gathered (tiny); K/V (+ inverse
key norms) are all-gathered within each 4-core batch group. Attention,
both projections and the full FFN run locally on the core's tokens, so
those two small collectives are the only cross-core traffic.

QKV / attn-out / FFN-out matmuls run in fp8(e4m3) DoubleRow perf mode
(two K-tiles per pass, 2x PE throughput); FFN-in stays bf16 to keep
the rel-err margin (fp8 there costs ~6e-3 extra). PSUM accumulates
fp32. fp8 weights are stored x64 so sigma~0.02 values sit in e4m3's
normal range; the 1/64 is folded into consumers (q/k l2-norms absorb
it for free, activation-scale operands and the gate broadcast rows
handle the rest). Measured rel err 1.18e-2 vs the 2e-2 gate.

Tricks (validated in sim probes + HW):
 - Q/K head features permuted host-side (evens then odds per head) so
   interleaved RoPE becomes two broadcast-multiplies against a 4-slot
   [cos,sin,-sin,cos] table plus one add, all bf16 (DVE 2x mode).
 - Matmul bias injected into PSUM with a K=1 ones-matmul; FFN biases
   ride activation-bias columns and a 1.0-activation row against a
   bias row stored in w_fo's K-padding (zero PE cost).
 - Logits computed transposed (PT[t,s]); two heads packed into the PE
   array concurrently via tile_position (K=64 row groups 0:64/64:128).
 - Softmax denominators from a ones-column appended to V (M=65 P@V);
   the denominator row is broadcast with a K=1 matmul issued at
   tile_position (64,0) - no cross-partition DMA round trip.
 - kT and V all-gathers are split per chunk so the kT gather (which
   gates the first logits) overlaps the v/q sections.
 - All transposes are batched multi-tile DmaTransposes (HWDGE triggers,
   ~0.6us each, are the scarce resource - DMA count is minimized
   everywhere: whole-chunk packs/recvs, single tokens load, per-si
   output stores).
 - Modulate streams alternate DVE/Pool to halve their serial tails.
"""

import sys

if "/opt/trn_rl_repo" not in sys.path:
    sys.path.insert(0, "/opt/trn_rl_repo")

import numpy as np
import ml_dtypes

import concourse.bass as bass
import concourse.bacc as bacc
import concourse.mybir as mybir
from concourse.tile import TileContext
from concourse import bass_utils

bf16 = mybir.dt.bfloat16
f32 = mybir.dt.float32
f8 = mybir.dt.float8e4
DR = mybir.MatmulPerfMode.DoubleRow
WSCALE = 64.0  # fp8 weight pre-scale; 1/WSCALE folded into consumers
AF = mybir.ActivationFunctionType
OP = mybir.AluOpType

FULL_CFG = dict(B=2, S=2048, D=1024, H=16, DH=64, INNER=2730, use_silu=True)
N_CORES = 8
GROUPS = [[0, 1, 2, 3], [4, 5, 6, 7]]


def derived(cfg):
    d = dict(cfg)
    B, S, D, H, DH, INNER = (cfg[k] for k in ("B", "S", "D", "H", "DH", "INNER"))
    d["S_LOC"] = S * B // N_CORES
    d["NSI"] = d["S_LOC"] // 128          # local s-tiles
    d["NFI"] = D // 128                   # feature tiles
    d["NT"] = S // 128                    # key-token tiles (full seq)
    d["NTL"] = d["S_LOC"] // 128          # local key-token tiles
    d["HP"] = H // 2                      # head pairs
    d["NII"] = (INNER + 127) // 128       # inner tiles (padded)
    d["IP"] = d["NII"] * 128              # padded inner dim
    d["MODSH"] = 6 * D // 4               # mod cols per core (4-way in group)
    # kv allgather: 4 chunks, one per group of H/4 heads. Per chunk:
    # kT tiles for HP/4 head-pairs (pre-scaled by 0.125/|k|) + v_aug cols.
    d["CH_HP"] = d["HP"] // 4             # head-pairs per chunk
    d["CH_H"] = H // 4                    # heads per chunk
    d["CH_KT"] = d["CH_HP"] * 128 * d["S_LOC"]
    d["CH_V"] = d["NTL"] * 128 * d["CH_H"] * 65
    d["CH_OFF_V"] = d["CH_KT"]
    d["NKV"] = d["CH_KT"] + d["CH_V"]
    return d


# ------------------------------------------------------------- host prep

def qk_perm(cfg):
    H, DH = cfg["H"], cfg["DH"]
    p = []
    for h in range(H):
        base = h * DH
        p += [base + i for i in range(0, DH, 2)]
        p += [base + i for i in range(1, DH, 2)]
    return np.array(p, dtype=np.int64)


def prep_inputs(cfg, inputs):
    """Full inputs -> 8 per-core input maps (host-side slicing only)."""
    c = derived(cfg)
    B, S, D, H, DH, INNER = (c[k] for k in ("B", "S", "D", "H", "DH", "INNER"))
    S_LOC, MODSH = c["S_LOC"], c["MODSH"]

    tokens = np.ascontiguousarray(inputs["tokens"], dtype=np.float32)
    cond = np.ascontiguousarray(inputs["condition"], dtype=np.float32)

    perm = qk_perm(cfg)
    w_qkv = np.asarray(inputs["w_qkv"], dtype=np.float32)
    b_qkv = np.asarray(inputs["b_qkv"], dtype=np.float32)
    w_qkv_p = w_qkv.copy()
    b_qkv_p = b_qkv.copy()
    w_qkv_p[:, 0:D] = w_qkv[:, 0:D][:, perm]
    w_qkv_p[:, D:2 * D] = w_qkv[:, D:2 * D][:, perm]
    b_qkv_p[0:D] = b_qkv[0:D][perm]
    b_qkv_p[D:2 * D] = b_qkv[D:2 * D][perm]

    wb = lambda a: np.ascontiguousarray(np.asarray(a, np.float32).astype(ml_dtypes.bfloat16))
    # fp8 weights are stored x64 so sigma~0.02 weights land in e4m3's
    # normal range (subnormals below 2^-6 would eat the mantissa); the
    # 1/64 is folded into downstream ops (q/k l2-norms absorb it free).
    w8 = lambda a: np.ascontiguousarray(
        (np.asarray(a, np.float32) * WSCALE).astype(ml_dtypes.float8_e4m3))
    fr = lambda a: np.ascontiguousarray(
        np.asarray(a, np.float32).reshape(1, -1).astype(ml_dtypes.bfloat16))

    NII, IP, NFI = c["NII"], c["IP"], c["NFI"]
    w_qkv_b = w8(w_qkv_p)
    w_ao_b = w8(inputs["w_attn_out"])
    # FFN-in: pad inner to IP and retile so each (sec, ii) tile-row block
    # is a [128, D] contiguous slab: row (sec*NII+ii)*128+p holds, for
    # every fi, w_fi[fi*128+p, sec*INNER + ii*128 + i] at col fi*128+i.
    w_fi_f = np.asarray(inputs["w_ffn_in"], np.float32)
    w_fi_pad = np.zeros((D, 2, IP), np.float32)
    w_fi_pad[:, 0, :INNER] = w_fi_f[:, :INNER]
    w_fi_pad[:, 1, :INNER] = w_fi_f[:, INNER:]
    w_fi_t = w_fi_pad.reshape(NFI, 128, 2, NII, 128).transpose(2, 3, 1, 0, 4)
    w_fi_b = wb(w_fi_t.reshape(2 * NII * 128, D))
    w_fo_pad = np.zeros((IP, D), np.float32)
    w_fo_pad[:INNER, :] = np.asarray(inputs["w_ffn_out"], np.float32)
    # FFN-out bias rides contraction row INNER (a zero-padding row whose
    # activation is memset to 1.0 in-kernel).
    w_fo_pad[INNER, :] = np.asarray(inputs["b_ffn_out"], np.float32)
    w_fo_b = w8(w_fo_pad)
    b_fi_pad = np.zeros((2, IP), np.float32)
    b_fi_f = np.asarray(inputs["b_ffn_in"], np.float32)
    b_fi_pad[0, :INNER] = b_fi_f[:INNER]
    b_fi_pad[1, :INNER] = b_fi_f[INNER:]
    # per-partition bias columns [128, 2*NII]: col sec*NII+ii row p holds
    # b_fi_pad[sec, ii*128+p]
    b_fi_cols = np.ascontiguousarray(
        b_fi_pad.reshape(2, NII, 128).transpose(2, 0, 1).reshape(128, 2 * NII)
        .astype(np.float32))
    w_mod = np.asarray(inputs["w_mod"], dtype=np.float32)
    b_mod = np.asarray(inputs["b_mod"], dtype=np.float32)

    half = DH // 2
    pos = np.arange(S, dtype=np.float32)
    inv_freq = 1.0 / (10000.0 ** (np.arange(half, dtype=np.float32) / half))
    theta = np.outer(pos, inv_freq)
    # 4-slot rope table: evens multiply [cos, sin], odds [-sin, cos];
    # summing the two products gives (re, ro) in one DVE add.
    cs_full = np.stack([np.cos(theta), np.sin(theta),
                        -np.sin(theta), np.cos(theta)],
                       axis=1).astype(ml_dtypes.bfloat16)

    in_maps = []
    for core in range(N_CORES):
        b, j = core // 4, core % 4
        r0 = j * S_LOC
        sh = slice((core % 4) * MODSH, (core % 4 + 1) * MODSH)
        in_maps.append({
            "tokens_c": tokens[b, r0:r0 + S_LOC, :],
            "cond_row": np.ascontiguousarray(cond[b:b + 1, :]),
            "cs_t": np.ascontiguousarray(cs_full[r0:r0 + S_LOC, :, :]),
            "w_qkv": w_qkv_b,
            "b_qkv": fr(b_qkv_p * WSCALE),
            "w_ao": w_ao_b,
            "b_ao": fr(np.asarray(inputs["b_attn_out"], np.float32) * WSCALE),
            "w_fi": w_fi_b,
            "b_fi_cols": b_fi_cols,
            "w_fo": w_fo_b,
            "w_mod_sh": wb(w_mod[:, sh]),
            "b_mod_sh": fr(b_mod[sh]),
        })
    return in_maps


# --------------------------------------------------------------- builder

def build_nc(cfg, local_collectives=False, repeat=1):
    c = derived(cfg)
    B, S, D, H, DH, INNER = (c[k] for k in ("B", "S", "D", "H", "DH", "INNER"))
    S_LOC, NSI, NFI, NT, NTL, HP = (
        c[k] for k in ("S_LOC", "NSI", "NFI", "NT", "NTL", "HP"))
    NII, IP, MODSH = c["NII"], c["IP"], c["MODSH"]
    NKV = c["NKV"]
    CH_HP, CH_H = c["CH_HP"], c["CH_H"]
    CH_OFF_V = c["CH_OFF_V"]
    half = DH // 2
    use_silu = cfg.get("use_silu", True)

    nc = bacc.Bacc("TRN2", num_devices=N_CORES)
    DP = nc.declare_dram_parameter
    tokens_c = DP("tokens_c", [S_LOC, D], f32, isOutput=False)
    cond_row = DP("cond_row", [1, D], f32, isOutput=False)
    cs_in = DP("cs_t", [S_LOC, 4, half], bf16, isOutput=False)
    w_qkv = DP("w_qkv", [D, 3 * D], f8, isOutput=False)
    b_qkv = DP("b_qkv", [1, 3 * D], bf16, isOutput=False)
    w_ao = DP("w_ao", [D, D], f8, isOutput=False)
    b_ao = DP("b_ao", [1, D], bf16, isOutput=False)
    w_fi = DP("w_fi", [2 * NII * 128, D], bf16, isOutput=False)
    b_fi_cols = DP("b_fi_cols", [128, 2 * NII], f32, isOutput=False)
    w_fo = DP("w_fo", [IP, D], f8, isOutput=False)
    w_mod_sh = DP("w_mod_sh", [D, MODSH], bf16, isOutput=False)
    b_mod_sh = DP("b_mod_sh", [1, MODSH], bf16, isOutput=False)
    out_c = DP("out", [S_LOC, D], f32, isOutput=True)

    warm_in = nc.dram_tensor("warm_in", [1, 128], f32)
    warm_out = nc.dram_tensor("warm_out", [4, 128], f32)
    mod_in = nc.dram_tensor("mod_in", [1, MODSH], f32)
    mod_out = nc.dram_tensor("mod_out", [4, MODSH], f32)
    CH_KT, CH_V = c["CH_KT"], c["CH_V"]
    kt_in = [nc.dram_tensor(f"kt_in{i}", [CH_KT], bf16) for i in range(4)]
    kt_out = [nc.dram_tensor(f"kt_out{i}", [4, CH_KT], bf16) for i in range(4)]
    v_in = [nc.dram_tensor(f"v_in{i}", [CH_V], bf16) for i in range(4)]
    v_out = [nc.dram_tensor(f"v_out{i}", [4, CH_V], bf16) for i in range(4)]

    from contextlib import ExitStack

    def _allgather(in_t, out_t, rows=4):
        """AllGather within the 4-core batch group, or (for single-core
        TimelineSim analysis) a local DMA stand-in with the same per-core
        receive volume."""
        if not local_collectives:
            nc.gpsimd.collective_compute(
                "AllGather", OP.bypass, replica_groups=GROUPS,
                ins=[in_t.ap()], outs=[out_t.ap()])
            return
        # Stand-in copy goes on the Pool (gpsimd) queue - where the real
        # collective trigger lives - so it neither serializes the SP
        # queue's packs/recvs nor blocks ACT work in the simulated
        # timeline. One stride-0-source DMA writes all group rows.
        src = in_t.ap()
        if len(src.shape) == 1:
            src = src.rearrange("(o n) -> o n", o=1)
        dst = out_t.ap()
        nc.scalar.dma_start(
            out=dst[0:rows, :], in_=src.to_broadcast((rows, src.shape[1])))

    tc = None

    def _emit(top):
        pool = lambda name, bufs, **kw: top.enter_context(
            tc.tile_pool(name=name, bufs=bufs, **kw))
        singles = pool("singles", 1)

        # tokens first: LN1 heads the longest dependency chain, so its
        # input DMAs go ahead of every weight load. Tiles stay resident
        # for the attention residual (saves the reload DMAs too).
        tok_pool = pool("tok", 1)
        tokb = tok_pool.tile([128, NSI, D], f32, name="tokb")
        nc.sync.dma_start(
            out=tokb,
            in_=tokens_c.ap().rearrange("(si p) d -> p si d", p=128))
        tok = [tokb[:, si, :] for si in range(NSI)]

        # long-lived pools, allocated up-front (pool alloc/release is LIFO)
        t2_pool = pool("t2", NSI)
        yT_pool = pool("yT", 1)
        act_pool = pool("act", (NII + 1) // 2)
        outT_pool = pool("outT", HP // 2)
        qT_pool = pool("qT", 1)
        xmT_pool = tc.alloc_tile_pool(name="xmT", bufs=NFI // 2)  # released after QKV

        # ---- constants
        ones1f = singles.tile([1, 128], f32)
        nc.vector.memset(ones1f, 1.0)
        onesP = singles.tile([128, 64], f32)
        nc.vector.memset(onesP, 1.0)
        ones1b = singles.tile([1, 128], bf16)
        nc.vector.memset(ones1b, 1.0)
        ones8row = singles.tile([1, 512], f8)
        nc.vector.memset(ones8row, 1.0)
        zero_col = singles.tile([128, 1], f32)
        nc.vector.memset(zero_col, 0.0)
        eps_col = singles.tile([128, 1], f32)
        nc.vector.memset(eps_col, 1e-5)
        eps12_col = singles.tile([128, 1], f32)
        nc.vector.memset(eps12_col, 1e-12)

        bmod_sb = singles.tile([1, MODSH], bf16)
        nc.sync.dma_start(out=bmod_sb, in_=b_mod_sh[:, :])

        ident = singles.tile([128, 128], bf16)
        from concourse.masks import make_identity
        make_identity(nc, ident)

        # warmup collective: absorbs the first-collective cold cost and
        # cross-core start skew while DMAs/LN run underneath.
        warm_sb = singles.tile([1, 128], f32)
        nc.vector.memset(warm_sb, 0.0)
        nc.sync.dma_start(out=warm_in[:, :], in_=warm_sb)
        _allgather(warm_in, warm_out)

        # ================= PHASE LN1 ==================================
        # Emitted before the mod phase so LN1's DVE work isn't queued
        # behind the mod-psum copies on the in-order DVE sequencer.
        xn_pool = tc.alloc_tile_pool(name="xn", bufs=NSI)
        xn_list = []
        with tc.tile_pool(name="ln1t", bufs=3) as ln1t:
            for si in range(NSI):
                t = tok[si]
                stats = ln1t.tile([128, D // 512, 6], f32, tag="stats")
                for g_ in range(D // 512):
                    nc.vector.bn_stats(out=stats[:, g_, :],
                                       in_=t[:, g_ * 512:(g_ + 1) * 512])
                mv = ln1t.tile([128, 2], f32, tag="mv")
                nc.vector.bn_aggr(out=mv, in_=stats)
                sd = ln1t.tile([128, 1], f32, tag="sd")
                nc.scalar.activation(out=sd, in_=mv[:, 1:2], func=AF.Sqrt,
                                     bias=eps_col[:, 0:1])
                rstd = ln1t.tile([128, 1], f32, tag="rstd")
                nc.vector.reciprocal(out=rstd, in_=sd)
                xn = xn_pool.tile([128, D], bf16, tag="xn", name="xn")
                nc.vector.tensor_scalar(out=xn, in0=t, scalar1=mv[:, 0:1],
                                        scalar2=rstd, op0=OP.subtract,
                                        op1=OP.mult)
                xn_list.append(xn)

        # ================= PHASE MOD: sharded AdaLN modulation ========
        modtmp_pool = tc.alloc_tile_pool(name="modtmp", bufs=1)
        condT = modtmp_pool.tile([128, NFI, 1], f32)
        nc.sync.dma_start(
            out=condT,
            in_=cond_row.ap().rearrange("o (kt kp) -> kp kt o", kp=128))
        sgc = modtmp_pool.tile([128, NFI], f32)
        nc.scalar.activation(out=sgc, in_=condT.rearrange("p k o -> p (k o)"),
                             func=AF.Sigmoid, bias=zero_col[:, 0:1])
        siluT = modtmp_pool.tile([128, NFI, 1], bf16)
        nc.vector.tensor_tensor(
            out=siluT, in0=condT,
            in1=sgc.rearrange("p (k o) -> p k o", o=1), op=OP.mult)

        modloc = modtmp_pool.tile([1, MODSH], f32)
        with tc.tile_pool(name="modw", bufs=2) as modw, \
             tc.tile_pool(name="modp", bufs=2, space="PSUM") as modp:
            n512 = MODSH // 512
            for nn in range(n512):
                wm = modw.tile([128, NFI, 512], bf16)
                nc.sync.dma_start(
                    out=wm,
                    in_=w_mod_sh.ap()[:, nn * 512:(nn + 1) * 512]
                    .rearrange("(kt kp) n -> kp kt n", kp=128))
                pm = modp.tile([1, 512], f32)
                nc.tensor.matmul(pm, ones1b[0:1, 0:1],
                                 bmod_sb[0:1, nn * 512:(nn + 1) * 512],
                                 start=True, stop=False)
                for kt in range(NFI):
                    nc.tensor.matmul(pm, siluT[:, kt, :], wm[:, kt, :],
                                     start=False, stop=(kt == NFI - 1))
                nc.vector.tensor_copy(out=modloc[0:1, nn * 512:(nn + 1) * 512],
                                      in_=pm)
        nc.sync.dma_start(out=mod_in[:, :], in_=modloc)
        _allgather(mod_in, mod_out)
        modtmp_pool.release()

        # ---- bias rows (after the mod collective is queued)
        bqkv_sb = singles.tile([1, 3 * D], bf16)
        nc.sync.dma_start(out=bqkv_sb, in_=b_qkv[:, :])
        bao_sb = singles.tile([1, D], bf16)
        nc.sync.dma_start(out=bao_sb, in_=b_ao[:, :])
        bficol_sb = singles.tile([128, 2 * NII], f32)
        nc.sync.dma_start(out=bficol_sb, in_=b_fi_cols[:, :])

        # rope tables (one batched DMA; slots [cos, sin, -sin, cos])
        cs_sb = singles.tile([128, NSI, 4, half], bf16)
        nc.sync.dma_start(
            out=cs_sb,
            in_=cs_in.ap().rearrange("(si p) c h -> p si c h", p=128))

        # mod columns [128, 48]: col k*NFI+fi holds mod[k*D+fi*128+p].
        modcols = singles.tile([128, 6 * NFI], f32)
        nc.sync.dma_start(
            out=modcols,
            in_=mod_out.ap().rearrange("g m -> (g m)")
            .rearrange("(j p) -> p j", p=128))
        for k in (1, 4):
            nc.vector.tensor_scalar_add(
                modcols[:, k * NFI:(k + 1) * NFI],
                modcols[:, k * NFI:(k + 1) * NFI], 1.0)
        # ============= PHASE XMT: transpose + modulate-to-fp8 =========
        # xm8b[:, fi, :] holds the modulated x^T for feature tile fi
        # in fp8 (DoubleRow pairs are adjacent fi). One batched
        # DmaTranspose per si (all 8 feature tiles in one HWDGE op),
        # then per-(si,fi) modulate so QKV(si) starts as soon as its
        # own column block is ready.
        xm8b = xmT_pool.tile([128, NFI, S_LOC], f8, name="xm8b")
        with tc.tile_pool(name="xtt", bufs=NSI) as xtt_pool:
            for si in range(NSI):
                xt = xtt_pool.tile([128, NFI, 128], bf16, tag="xt",
                                   name="xt")
                nc.sync.dma_start(out=xt, in_=xn_list[si],
                                  transpose=True)
                for fi in range(NFI):
                    # on Pool: DVE is saturated by LN1/rope in this window
                    eng = nc.gpsimd if si % 2 else nc.vector
                    eng.tensor_scalar(
                        out=xm8b[:, fi, si * 128:(si + 1) * 128],
                        in0=xt[:, fi, :],
                        scalar1=modcols[:, 1 * NFI + fi:1 * NFI + fi + 1],
                        scalar2=modcols[:, 0 * NFI + fi:0 * NFI + fi + 1],
                        op0=OP.mult, op1=OP.add)
        xn_pool.release()

        # ================= PHASE QKV ==================================
        # section order: k (2,3), v (4,5), q (0,1) so the kv allgather
        # can launch while q-side work proceeds.
        qkv_scope = ExitStack()
        spool = lambda name, bufs, **kw: qkv_scope.enter_context(
            tc.tile_pool(name=name, bufs=bufs, **kw))
        knat_pool = spool("knat", NSI)
        qnat_pool = spool("qnat", NSI)
        vaug_pool = spool("vaug", 1)
        knat = [knat_pool.tile([128, D], bf16, tag="knat", name="knat") for _ in range(NSI)]
        qnat = [qnat_pool.tile([128, D], bf16, tag="qnat", name="qnat") for _ in range(NSI)]
        vaug = vaug_pool.tile([128, NSI, H, 65], bf16, name="vaug")
        nc.vector.memset(vaug[:, :, :, 64:65], 1.0)

        def rope_norm(ps, si, dst, want_scale_q):
            """ps: psum [128, 1024] view (16 heads); rope+norm -> dst bf16.
            want_scale_q: True -> scale by 1/|q|; else by 0.125/|k|.

            Copies PSUM to bf16 on ACT, then two broadcast-multiplies
            against the 4-slot [cos,sin,-sin,cos] table and one add (all
            bf16 -> DVE 2x mode); per-head sum-of-squares reduce runs on
            the otherwise idle Pool engine."""
            qkb = rtmp.tile([128, 16, 2, half], bf16, tag="qkb")
            nc.scalar.activation(
                out=qkb, in_=ps.rearrange("p (h e j) -> p h e j", h=16, e=2),
                func=AF.Identity, bias=zero_col[:, 0:1])
            ttE = rtmp.tile([128, 16, 2, half], bf16, tag="ttE")
            ttO = rtmp.tile([128, 16, 2, half], bf16, tag="ttO")
            nc.vector.tensor_tensor(
                out=ttE,
                in0=qkb[:, :, 0:1, :].to_broadcast((128, 16, 2, half)),
                in1=cs_sb[:, si:si + 1, 0:2, :]
                .to_broadcast((128, 16, 2, half)), op=OP.mult)
            nc.vector.tensor_tensor(
                out=ttO,
                in0=qkb[:, :, 1:2, :].to_broadcast((128, 16, 2, half)),
                in1=cs_sb[:, si:si + 1, 2:4, :]
                .to_broadcast((128, 16, 2, half)), op=OP.mult)
            dv = dst[:, :].rearrange("p (h e j) -> p h e j", h=16, e=2)
            nc.vector.tensor_tensor(out=dv, in0=ttE, in1=ttO, op=OP.add)
            # per-head sum of squares
            sq = rtmp.tile([128, D], bf16, tag="sq")
            nc.scalar.activation(out=sq, in_=dst[:, :],
                                 func=AF.Square, bias=zero_col[:, 0:1])
            ssq = rtmp.tile([128, 16], f32, tag="ssq")
            nc.vector.tensor_reduce(out=ssq,
                                    in_=sq.rearrange("p (h d) -> p h d", h=16),
                                    axis=mybir.AxisListType.X, op=OP.add)
            sdv = rtmp.tile([128, 16], f32, tag="sdv")
            if want_scale_q:
                nc.scalar.activation(out=sdv, in_=ssq, func=AF.Sqrt,
                                     bias=eps12_col[:, 0:1])
            else:
                nc.scalar.activation(out=sdv, in_=ssq, func=AF.Sqrt,
                                     scale=float(DH), bias=eps12_col[:, 0:1])
            rr = rtmp.tile([128, 16], f32, tag="rr")
            nc.vector.reciprocal(out=rr, in_=sdv)
            dall = dst[:, :].rearrange("p (h d) -> p h d", h=16)
            nc.vector.tensor_tensor(
                out=dall, in0=dall,
                in1=rr.rearrange("p (h o) -> p h o", o=1)
                .to_broadcast((128, 16, DH)), op=OP.mult)

        kTl_pool = spool("kTl", 1)
        kTlb = kTl_pool.tile([128, HP, S_LOC], bf16, name="kTlb")
        qTb = qT_pool.tile([128, HP, S_LOC], bf16, name="qTb")

        def ship_kt(ch):
            """Pack chunk ch's kT head-pairs and launch its AllGather.
            Shipped before the v-section runs so the kT gather (which
            gates the first attention logits) overlaps the v/q work."""
            nc.sync.dma_start(
                out=kt_in[ch].ap()[:]
                .rearrange("(k p s) -> p k s", p=128, k=CH_HP),
                in_=kTlb[:, ch * CH_HP:(ch + 1) * CH_HP, :])
            _allgather(kt_in[ch], kt_out[ch])

        def ship_v(ch):
            h0 = ch * CH_H
            nc.sync.dma_start(
                out=v_in[ch].ap()[:]
                .rearrange("(si p x) -> p si x", p=128, si=NSI),
                in_=vaug[:, :, h0:h0 + CH_H, :]
                .rearrange("p si h x -> p si (h x)"),
            )
            _allgather(v_in[ch], v_out[ch])

        with tc.tile_pool(name="wqkv", bufs=2) as wqkv_pool, \
             tc.tile_pool(name="qkvp", bufs=4, space="PSUM") as qkvp, \
             tc.tile_pool(name="rtmp", bufs=3) as rtmp:
            for step in ("k", "ship_kt", "v", "ship_v", "q", "qT"):
                if step == "ship_kt":
                    # batched kT transposes: one HWDGE op per si covers
                    # all head pairs.
                    for si in range(NSI):
                        nc.sync.dma_start(
                            out=kTlb[:, :, si * 128:(si + 1) * 128],
                            in_=knat[si], transpose=True)
                    for ch in range(4):
                        ship_kt(ch)
                    continue
                if step == "ship_v":
                    for ch in range(4):
                        ship_v(ch)
                    continue
                if step == "qT":
                    # batched DMA transposes (no PE/DVE involvement)
                    for si in range(NSI):
                        nc.sync.dma_start(
                            out=qTb[:, :, si * 128:(si + 1) * 128],
                            in_=qnat[si], transpose=True)
                    continue
                nA = {"k": 2, "v": 4, "q": 0}[step]
                if step == "k":
                    wsec_pre = {}
                    for st2, nA2 in (("k", 2), ("v", 4)):
                        w2 = wqkv_pool.tile([128, NFI, 1024], f8, tag="w",
                                            name="wsec")
                        nc.sync.dma_start(
                            out=w2,
                            in_=w_qkv.ap()[:, nA2 * 512:(nA2 + 2) * 512]
                            .rearrange("(fo p) n2 -> p fo n2", p=128))
                        wsec_pre[st2] = w2
                if step in wsec_pre:
                    wsec = wsec_pre[step]
                else:
                    wsec = wqkv_pool.tile([128, NFI, 1024], f8, tag="w",
                                          name="wsec")
                    nc.sync.dma_start(
                        out=wsec,
                        in_=w_qkv.ap()[:, nA * 512:(nA + 2) * 512]
                        .rearrange("(fo p) n2 -> p fo n2", p=128))
                for si in range(NSI):
                    ps = qkvp.tile([128, 2, 512], f32, tag="qkv")
                    for h_ in range(2):
                        nc.tensor.matmul(
                            ps[:, h_, :], ones1b,
                            bqkv_sb[0:1, (nA + h_) * 512:(nA + h_ + 1) * 512],
                            start=True, stop=False)
                    for fp in range(NFI // 2):
                        for h_ in range(2):
                            nc.tensor.matmul(
                                ps[:, h_, :],
                                xm8b[:, 2 * fp:2 * fp + 2,
                                     si * 128:(si + 1) * 128],
                                wsec[:, 2 * fp:2 * fp + 2,
                                     h_ * 512:(h_ + 1) * 512],
                                start=False, stop=(fp == NFI // 2 - 1),
                                perf_mode=DR)
                    psf = ps.rearrange("p a b -> p (a b)")
                    if step == "k":
                        rope_norm(psf, si, knat[si], False)
                    elif step == "q":
                        rope_norm(psf, si, qnat[si], True)
                    else:
                        # v: undo the x64 weight pre-scale on the way out
                        # of PSUM (q/k norms absorb it instead).
                        nc.scalar.activation(
                            out=vaug[:, si, :, 0:64],
                            in_=psf.rearrange("p (h d) -> p h d", h=16),
                            func=AF.Identity, scale=1.0 / WSCALE,
                            bias=zero_col[:, 0:1])
        qkv_scope.close()
        xmT_pool.release()

        # gate-row PE broadcasts. Emitted here (not before QKV): on the
        # in-order PE queue they wait for the full mod result, and the
        # gates aren't consumed until the residuals.
        garow = singles.tile([1, D], f32)
        gmrow = singles.tile([1, D], f32)
        ga_r, ga_c = divmod(2 * D, MODSH)
        gm_r, gm_c = divmod(5 * D, MODSH)
        assert ga_c + D <= MODSH and gm_c + D <= MODSH
        nc.sync.dma_start(out=garow, in_=mod_out[ga_r:ga_r + 1, ga_c:ga_c + D])
        nc.sync.dma_start(out=gmrow, in_=mod_out[gm_r:gm_r + 1, gm_c:gm_c + D])
        garep = singles.tile([128, D], bf16)
        gmrep = singles.tile([128, D], bf16)
        # broadcast gate/WSCALE: the 1/64 undoes the fp8 weight pre-scale
        # of the AO / FFN-out PSUM these gates multiply.
        inv64f = singles.tile([1, 128], f32)
        nc.vector.memset(inv64f, 1.0 / WSCALE)
        with tc.tile_pool(name="gatep", bufs=2, space="PSUM") as gatep:
            for row, rep in ((garow, garep), (gmrow, gmrep)):
                for nn in range(D // 512):
                    gp = gatep.tile([128, 512], f32)
                    nc.tensor.matmul(gp, inv64f,
                                     row[0:1, nn * 512:(nn + 1) * 512],
                                     start=True, stop=True)
                    nc.vector.tensor_copy(
                        out=rep[:, nn * 512:(nn + 1) * 512], in_=gp)

        # FFN-in weight pool hoisted here so the first tiles prefetch
        # during attention (released after FFN-in -> alloc before wao).
        wfi_pool = tc.alloc_tile_pool(name="wfi", bufs=6)
        wfi_pre = {}
        for ii0 in range(2):
            wv0 = wfi_pool.tile([128, NFI, 128], bf16, tag="wfi", name="wfiv")
            nc.sync.dma_start(
                out=wv0,
                in_=w_fi.ap()[ii0 * 128:(ii0 + 1) * 128, :]
                .rearrange("p (fi i) -> p fi i", i=128))
            wg0 = wfi_pool.tile([128, NFI, 128], bf16, tag="wfi", name="wfig")
            nc.sync.dma_start(
                out=wg0,
                in_=w_fi.ap()[(NII + ii0) * 128:(NII + ii0 + 1) * 128, :]
                .rearrange("p (fi i) -> p fi i", i=128))
            wfi_pre[ii0] = (wv0, wg0)

        # ================= PHASE ATTENTION ============================
        wao_pool = tc.alloc_tile_pool(name="wao", bufs=2)
        wao_t = []
        for fn in range(D // 512):
            w_ = wao_pool.tile([128, NFI, 512], f8, tag="wao", name="wao")
            nc.sync.dma_start(
                out=w_,
                in_=w_ao.ap()[:, fn * 512:(fn + 1) * 512]
                .rearrange("(ct p) f -> p ct f", p=128))
            wao_t.append(w_)
        # o8[j][:, t, :]: attention output^T for head-pair 2j+t, fp8,
        # packed for the DoubleRow out-projection.
        o8 = [outT_pool.tile([128, 2, S_LOC], f8, tag="outT", name="o8")
              for _ in range(HP // 2)]
        with tc.tile_pool(name="vfch", bufs=2) as vfch, \
             tc.tile_pool(name="kTf", bufs=4) as kTf_pool, \
             tc.tile_pool(name="pt", bufs=6) as pt_pool, \
             tc.tile_pool(name="pslog", bufs=2, space="PSUM") as pslog, \
             tc.tile_pool(name="psO", bufs=3, space="PSUM") as psO_pool, \
             tc.tile_pool(name="psbc", bufs=1, space="PSUM") as psbc, \
             tc.tile_pool(name="dtmp", bufs=2) as dtmp:
            for ch in range(4):
                # kT first: the logits matmuls need only kT + qT, so they
                # can start while the (bigger) V tiles are still landing.
                # whole-chunk recvs: one DMA for both head-pairs' kT and
                # one for all of V (HWDGE triggers are the scarce resource).
                kTfb = kTf_pool.tile([128, CH_HP, S], bf16, tag="kTf",
                                     name="kTfb")
                for k_ in range(CH_HP):
                    nc.sync.dma_start(
                        out=kTfb[:, k_, :].rearrange("p (g s) -> p g s", g=4),
                        in_=kt_out[ch].ap()[:, k_ * 128 * S_LOC:
                                            (k_ + 1) * 128 * S_LOC]
                        .rearrange("g (p s) -> p g s", p=128))
                kTf_ch = [kTfb[:, k_, :] for k_ in range(CH_HP)]
                vfb = vfch.tile([128, NT, CH_H, 65], bf16, tag="vf",
                                name="vfb")
                nc.sync.dma_start(
                    out=vfb.rearrange("p t h x -> p t (h x)"),
                    in_=v_out[ch].ap()
                    .rearrange("g (jj p x) -> p (g jj) x", p=128, jj=NTL))
                vf = [vfb[:, tj, :, :] for tj in range(NT)]
                for k_ in range(CH_HP):
                    hp = ch * CH_HP + k_
                    hA, hB = 2 * k_, 2 * k_ + 1
                    kTf = kTf_ch[k_]
                    psOA = psO_pool.tile([65, S_LOC], f32, tag="psO")
                    psOB = psO_pool.tile([65, S_LOC], f32, tag="psO")
                    # software pipeline: P@V for tile tj-1 is emitted after
                    # the logits matmuls of tile tj, so the PE never sits
                    # in-queue behind a wait on exp(tj).
                    pending = None
                    for tj in range(NT):
                        psP = pslog.tile([128, 2, S_LOC], f32, tag="lg")
                        nc.tensor.matmul(psP[:, 0, :],
                                         kTf[0:64, tj * 128:(tj + 1) * 128],
                                         qTb[0:64, hp, :], start=True,
                                         stop=True, tile_position=(0, 0))
                        nc.tensor.matmul(psP[:, 1, :],
                                         kTf[64:128, tj * 128:(tj + 1) * 128],
                                         qTb[64:128, hp, :], start=True,
                                         stop=True, tile_position=(64, 0))
                        pt = pt_pool.tile([128, 2, S_LOC], bf16, tag="pt")
                        nc.scalar.activation(out=pt, in_=psP, func=AF.Exp,
                                             bias=zero_col[:, 0:1])
                        if pending is not None:
                            ptj, ppt = pending
                            nc.tensor.matmul(psOA, vf[ptj][:, hA, :],
                                             ppt[:, 0, :], start=(ptj == 0),
                                             stop=False,
                                             skip_group_check=True)
                            nc.tensor.matmul(psOB, vf[ptj][:, hB, :],
                                             ppt[:, 1, :], start=(ptj == 0),
                                             stop=False,
                                             skip_group_check=True)
                        pending = (tj, pt)
                    ptj, ppt = pending
                    nc.tensor.matmul(psOA, vf[ptj][:, hA, :], ppt[:, 0, :],
                                     start=False, stop=True,
                                     skip_group_check=True)
                    nc.tensor.matmul(psOB, vf[ptj][:, hB, :], ppt[:, 1, :],
                                     start=False, stop=True,
                                     skip_group_check=True)
                    # normalize: rows 0:64 / denom row 64. Broadcast the
                    # raw denominator row with a K=1 PE matmul issued at
                    # tile_position (64,0) - reads the row in place at
                    # partition 64, no DMA round trip - then take the
                    # reciprocal across 64 lanes (a [1,512] reciprocal
                    # runs on a single DVE lane - 5x slower).
                    for head_i, psO in ((0, psOA), (1, psOB)):
                        d64 = dtmp.tile([128, S_LOC], f32, tag="d64")
                        nc.vector.tensor_copy(out=d64[64:65, :],
                                              in_=psO[64:65, :])
                        bc = psbc.tile([64, S_LOC], f32, tag="bc")
                        nc.tensor.matmul(bc, onesP[64:65, 0:64],
                                         d64[64:65, :], start=True,
                                         stop=True, tile_position=(64, 0))
                        bcs = dtmp.tile([64, S_LOC], f32, tag="bcs")
                        nc.vector.reciprocal(out=bcs, in_=bc)
                        if head_i == 0:
                            nc.vector.tensor_tensor(
                                out=o8[hp // 2][0:64, hp % 2, :],
                                in0=psO[0:64, :], in1=bcs, op=OP.mult)
                        else:
                            tmpB = dtmp.tile([64, S_LOC], f8, tag="tmpB")
                            nc.vector.tensor_tensor(out=tmpB,
                                                    in0=psO[0:64, :],
                                                    in1=bcs, op=OP.mult)
                            nc.sync.dma_start(
                                out=o8[hp // 2][64:128, hp % 2, :],
                                in_=tmpB)

        # ========== PHASE OUT-PROJ + RESIDUAL + LN2 + yT (fused) ======
        t2 = [t2_pool.tile([128, D], f32, tag="t2", name="t2") for _ in range(NSI)]
        ybig = yT_pool.tile([128, NFI, S_LOC], bf16, name="ybig")
        yT = [ybig[:, fi, :] for fi in range(NFI)]
        with tc.tile_pool(name="aop", bufs=2, space="PSUM") as aop, \
             tc.tile_pool(name="aot", bufs=4) as aot, \
             tc.tile_pool(name="ln2t", bufs=3) as ln2t, \
             tc.tile_pool(name="yn", bufs=2) as yn_pool:
            for si in range(NSI):
                for fn in range(D // 512):
                    ps = aop.tile([128, 512], f32, tag="ao")
                    nc.tensor.matmul(ps, ones1b,
                                     bao_sb[0:1, fn * 512:(fn + 1) * 512],
                                     start=True, stop=False)
                    for j in range(HP // 2):
                        nc.tensor.matmul(ps,
                                         o8[j][:, :, si * 128:(si + 1) * 128],
                                         wao_t[fn][:, 2 * j:2 * j + 2, :],
                                         start=False, stop=(j == HP // 2 - 1),
                                         perf_mode=DR)
                    tmp = aot.tile([128, 512], f32, tag="aotmp")
                    nc.vector.tensor_tensor(
                        out=tmp, in0=ps,
                        in1=garep[:, fn * 512:(fn + 1) * 512], op=OP.mult)
                    nc.vector.tensor_tensor(
                        out=t2[si][:, fn * 512:(fn + 1) * 512], in0=tmp,
                        in1=tok[si][:, fn * 512:(fn + 1) * 512], op=OP.add)
                # LN2 for this s-tile, immediately
                stats = ln2t.tile([128, D // 512, 6], f32, tag="stats2")
                for g_ in range(D // 512):
                    nc.vector.bn_stats(out=stats[:, g_, :],
                                       in_=t2[si][:, g_ * 512:(g_ + 1) * 512])
                mv = ln2t.tile([128, 2], f32, tag="mv2")
                nc.vector.bn_aggr(out=mv, in_=stats)
                sd = ln2t.tile([128, 1], f32, tag="sd2")
                nc.scalar.activation(out=sd, in_=mv[:, 1:2], func=AF.Sqrt,
                                     bias=eps_col[:, 0:1])
                rstd = ln2t.tile([128, 1], f32, tag="rstd2")
                nc.vector.reciprocal(out=rstd, in_=sd)
                yn = yn_pool.tile([128, D], bf16, tag="yn")
                nc.vector.tensor_scalar(out=yn, in0=t2[si], scalar1=mv[:, 0:1],
                                        scalar2=rstd, op0=OP.subtract,
                                        op1=OP.mult)
                nc.sync.dma_start(
                    out=ybig[:, :, si * 128:(si + 1) * 128],
                    in_=yn, transpose=True)
            for fi in range(NFI):
                # alternate DVE/Pool: halves the serial modulate tail
                # between LN2 and the first FFN matmul
                eng = nc.gpsimd if fi % 2 else nc.vector
                eng.tensor_scalar(
                    out=yT[fi], in0=yT[fi],
                    scalar1=modcols[:, 4 * NFI + fi:4 * NFI + fi + 1],
                    scalar2=modcols[:, 3 * NFI + fi:3 * NFI + fi + 1],
                    op0=OP.mult, op1=OP.add)
        wao_pool.release()

        # ================= PHASE FFN-IN + SWIGLU ======================
        wfo_pool = tc.alloc_tile_pool(name="wfo", bufs=2)
        wfo_t = []
        # act8[jj][:, t, :]: swiglu activations for inner tile 2*jj+t in
        # fp8, DoubleRow-packed for FFN-out. Gate bias rides the Silu
        # activation's bias column; value bias is added on the ACT engine
        # (Identity+bias); FFN-out bias rides contraction row INNER via a
        # 1.0 here (see prep_inputs).
        act8 = [act_pool.tile([128, 2, S_LOC], f8, tag="act", name="act8")
                for _ in range((NII + 1) // 2)]
        with tc.tile_pool(name="fip", bufs=4, space="PSUM") as fip, \
             tc.tile_pool(name="fit", bufs=3) as fit:
            for ii in range(NII):
                if ii in wfi_pre:
                    wv, wg = wfi_pre[ii]
                else:
                    wv = wfi_pool.tile([128, NFI, 128], bf16, tag="wfi",
                                       name="wfiv")
                    nc.sync.dma_start(
                        out=wv,
                        in_=w_fi.ap()[ii * 128:(ii + 1) * 128, :]
                        .rearrange("p (fi i) -> p fi i", i=128))
                    wg = wfi_pool.tile([128, NFI, 128], bf16, tag="wfi",
                                       name="wfig")
                    nc.sync.dma_start(
                        out=wg,
                        in_=w_fi.ap()[(NII + ii) * 128:(NII + ii + 1) * 128, :]
                        .rearrange("p (fi i) -> p fi i", i=128))
                if ii == 1:
                    # prefetch FFN-out weights once the first FFN-in tiles
                    # are in flight (not before: they'd queue ahead).
                    for fn in range(D // 512):
                        w_ = wfo_pool.tile([128, NII, 512], f8,
                                           tag="wfo", name="wfo")
                        nc.sync.dma_start(
                            out=w_,
                            in_=w_fo.ap()[:, fn * 512:(fn + 1) * 512]
                            .rearrange("(ii2 p) f -> p ii2 f", p=128))
                        wfo_t.append(w_)
                psv = fip.tile([128, S_LOC], f32, tag="fiv")
                psg = fip.tile([128, S_LOC], f32, tag="fig")
                for fi in range(NFI):
                    nc.tensor.matmul(psv, wv[:, fi, :], yT[fi],
                                     start=(fi == 0), stop=(fi == NFI - 1))
                    nc.tensor.matmul(psg, wg[:, fi, :], yT[fi],
                                     start=(fi == 0), stop=(fi == NFI - 1))
                sl = fit.tile([128, S_LOC], bf16, tag="sl")
                nc.scalar.activation(out=sl, in_=psg, func=AF.Silu,
                                     bias=bficol_sb[:, NII + ii:NII + ii + 1])
                # act = (value + bias) * silu(gate) fused in one DVE op
                nc.vector.scalar_tensor_tensor(
                    out=act8[ii // 2][:, ii % 2, :], in0=psv,
                    scalar=bficol_sb[:, ii:ii + 1], in1=sl,
                    op0=OP.add, op1=OP.mult)
            # 1.0 into the activation row for contraction row INNER so the
            # FFN-out bias (stored in w_fo row INNER) is added for free.
            # (DMA, not memset: engines can't address a single partition at
            # an unaligned base.)
            bii, bp = INNER // 128, INNER % 128
            nc.sync.dma_start(out=act8[bii // 2][bp:bp + 1, bii % 2, :],
                              in_=ones8row[0:1, 0:S_LOC])

        # ================= PHASE FFN-OUT + FINAL ======================
        with tc.tile_pool(name="fop", bufs=2, space="PSUM") as fop, \
             tc.tile_pool(name="fot", bufs=4) as fot:
            for si in range(NSI):
                osb = fot.tile([128, D], f32, tag="osb")
                for fn in range(D // 512):
                    ps = fop.tile([128, 512], f32, tag="fo")
                    for jj in range(NII // 2):
                        nc.tensor.matmul(
                            ps, act8[jj][:, :, si * 128:(si + 1) * 128],
                            wfo_t[fn][:, 2 * jj:2 * jj + 2, :],
                            start=(jj == 0), stop=(jj == NII // 2 - 1),
                            perf_mode=DR)
                    tmp = fot.tile([128, 512], f32, tag="fotmp")
                    nc.vector.tensor_tensor(
                        out=tmp, in0=ps,
                        in1=gmrep[:, fn * 512:(fn + 1) * 512], op=OP.mult)
                    nc.vector.tensor_tensor(
                        out=osb[:, fn * 512:(fn + 1) * 512], in0=tmp,
                        in1=t2[si][:, fn * 512:(fn + 1) * 512], op=OP.add)
                nc.sync.dma_start(
                    out=out_c[si * 128:(si + 1) * 128, :], in_=osb)
        wfo_pool.release()
        wfi_pool.release()

    with TileContext(nc) as _tc:
        tc = _tc
        for _rep in range(repeat):
            with ExitStack() as top:
                _emit(top)
    nc.compile()
    return nc


# --------------------------------------------------------------- entry

_NC_CACHE = {}


def _get_nc(key, cfg):
    if key not in _NC_CACHE:
        _NC_CACHE[key] = build_nc(cfg)
    return _NC_CACHE[key]


def kernel(**inputs) -> np.ndarray:
    cfg = FULL_CFG
    c = derived(cfg)
    nc = _get_nc("full", cfg)
    in_maps = prep_inputs(cfg, inputs)
    res = bass_utils.run_bass_kernel_spmd(
        nc, in_maps, core_ids=list(range(N_CORES)))
    B, S, D, S_LOC = c["B"], c["S"], c["D"], c["S_LOC"]
    out = np.empty((B, S, D), dtype=np.float32)
    for core in range(N_CORES):
        b, j = core // 4, core % 4
        out[b, j * S_LOC:(j + 1) * S_LOC, :] = res.results[core]["out"]
    return out

